# revision 2
# baseline (speedup 1.0000x reference)
"""Trainium2 Bass kernel for nn_KmerEmbed: conv1d(one-hot kmer filters) + relu + window-sum.

Computes, for seqs (32,32,30,21), weight (8000,20,3), bias (8000,):
  out[n,m,f] = sum_l relu( sum_{a,j} seqs[n,m,l+j,a(<20)]*weight[f,a,j] + bias[f] )
with l over the 28 valid conv positions; returns (32,32,8000) float32.

Strategy (8 NeuronCores, data-parallel over the 1024 flattened rows, 128 rows/core):
  - im2col on host: X[(j,a)+bias_row, tile, (n4,l28)] -> stationary operand of a
    K=61 matmul streamed against the replicated filter matrix Wt (61,8000) in
    float32r (1 cycle/row at N>=256, ~1e-4 rel precision).
  - conv tiles are packed in pairs into PE row-groups (partitions 0-60 / 64-124)
    so two matmuls stream concurrently.
  - relu(conv) evaluated from PSUM by ScalarE (activation Relu) and VectorE
    (tensor_scalar max) in parallel, written to SBUF as float16.
  - window-sum via a second matmul with 0/1 selection matrices G (112,32) in
    fp16, one PE column-group per 32-row output block; 8 tiles accumulate into
    each column group of a single (128, chunk) PSUM tile, so the final
    PSUM->SBUF copy covers all 128 partitions at once.
  - staging (128,8000) in SBUF, contiguous DMA to DRAM.
"""

import os
import sys

import numpy as np
from numpy.lib.stride_tricks import sliding_window_view

for _p in ("/opt/trn_rl_repo", "/root/.axon_site/_ro/trn_rl_repo"):
    if os.path.isdir(_p) and _p not in sys.path:
        sys.path.insert(0, _p)

import concourse.bacc as bacc
import concourse.mybir as mybir
from concourse.tile import TileContext
from concourse.bass_utils import run_bass_kernel_spmd

# problem sizes (hardcoded per spec)
N_, M_, L_, B_ = 32, 32, 30, 21
A_, K_ = 20, 3
F_ = 8000
NM = N_ * M_              # 1024
CORES = 8
NMC = NM // CORES         # 128 rows per core
LOUT = L_ - K_ + 1        # 28
NMG = 4                   # rows per conv tile
MT = NMG * LOUT           # 112 psum partitions per conv tile
NT = NMC // NMG           # 32 tiles per core
KC = A_ * K_ + 1          # 61 = 60 + bias row
FCH = 500                 # matmul free-dim chunk (one PSUM bank)
BIG = 1000                # relu/copy chunk (two banks)
NBIG = F_ // BIG          # 8

_f32r = mybir.dt.float32r
_f32 = mybir.dt.float32
_f16 = mybir.dt.float16

_cached_nc = None


def _build_program():
    nc = bacc.Bacc("TRN2", target_bir_lowering=False, debug=False,
                   num_devices=CORES)
    xin_d = nc.declare_dram_parameter("xin", [128, NT // 2 * MT], _f32r,
                                      isOutput=False)
    wt_d = nc.declare_dram_parameter("wt", [128, F_], _f32r, isOutput=False)
    g_d = nc.declare_dram_parameter("g", [MT, 8 * 32], _f16, isOutput=False)
    out_d = nc.declare_dram_parameter("out", [NMC, F_], _f32, isOutput=True)

    relu_fn = mybir.ActivationFunctionType.Relu
    max_op = mybir.AluOpType.max

    with TileContext(nc) as tc:
        with tc.tile_pool(name="const", bufs=1) as cpool, \
             tc.tile_pool(name="rbuf", bufs=18) as rpool, \
             tc.tile_pool(name="stage", bufs=1) as spool, \
             tc.tile_pool(name="pconv", bufs=3, space="PSUM") as pconv, \
             tc.tile_pool(name="psum", bufs=1, space="PSUM") as psump:
            xin_sb = cpool.tile([128, NT // 2 * MT], _f32r)
            wt_sb = cpool.tile([128, F_], _f32r)
            g_sb = cpool.tile([MT, 8 * 32], _f16)
            stage = spool.tile([NMC, F_], _f32)
            nc.sync.dma_start(out=xin_sb[:], in_=xin_d[:])
            nc.sync.dma_start(out=g_sb[:], in_=g_d[:])
            for i in range(4):
                s = slice(i * (F_ // 4), (i + 1) * (F_ // 4))
                nc.sync.dma_start(out=wt_sb[:, s], in_=wt_d[:, s])

            sum_order = [g0 * 8 + i for i in range(8) for g0 in range(4)]
            first_in_group = set(range(0, NT, 8))

            for c in range(NBIG):
                fsl = slice(c * BIG, (c + 1) * BIG)
                rtiles = {}
                for p in range(NT // 2):
                    pc_e = pconv.tile([MT, 1024], _f32, tag="pc")
                    pc_o = pconv.tile([MT, 1024], _f32, tag="pc")
                    for h in range(2):
                        fs = slice(c * BIG + h * FCH, c * BIG + (h + 1) * FCH)
                        hs = slice(h * 512, h * 512 + FCH)
                        nc.tensor.matmul(
                            out=pc_e[:, hs],
                            lhsT=xin_sb[0:KC, p * MT:(p + 1) * MT],
                            rhs=wt_sb[0:KC, fs], start=True, stop=True)
                        nc.tensor.matmul(
                            out=pc_o[:, hs],
                            lhsT=xin_sb[64:64 + KC, p * MT:(p + 1) * MT],
                            rhs=wt_sb[64:64 + KC, fs], start=True, stop=True)
                    r_e = rpool.tile([MT, 1024], _f16, tag="re")
                    r_o = rpool.tile([MT, 1024], _f16, tag="ro")
                    nc.scalar.activation(out=r_e[:, 0:1012], in_=pc_e[:, 0:1012],
                                         func=relu_fn)
                    nc.vector.tensor_scalar(out=r_o[:, 0:1012], in0=pc_o[:, 0:1012],
                                            scalar1=0.0, scalar2=None,
                                            op0=max_op)
                    rtiles[2 * p] = r_e
                    rtiles[2 * p + 1] = r_o
                ps = psump.tile([128, 1024], _f32)
                for h in range(2):
                    hs = slice(h * 512, h * 512 + FCH)
                    for t in sum_order:
                        grp = t // 8
                        oi = t % 8
                        nc.tensor.matmul(
                            out=ps[32 * grp:32 * grp + 32, hs],
                            lhsT=g_sb[:, 32 * oi:32 * oi + 32],
                            rhs=rtiles[t][:, hs],
                            start=(t in first_in_group),
                            stop=(t == sum_order[-1]),
                            skip_group_check=True,
                            tile_position=(0, 32 * grp))
                for h in range(2):
                    nc.vector.tensor_copy(
                        out=stage[:, c * BIG + h * FCH:c * BIG + (h + 1) * FCH],
                        in_=ps[:, h * 512:h * 512 + FCH])
                if c % 2 == 1:
                    osl = slice((c - 1) * BIG, (c + 1) * BIG)
                    nc.sync.dma_start(out=out_d[:, osl], in_=stage[:, osl])

    nc.compile()
    return nc


def _get_program():
    global _cached_nc
    if _cached_nc is None:
        _cached_nc = _build_program()
    return _cached_nc


def _host_prep(seqs, weight, bias):
    s = np.asarray(seqs, np.float32).reshape(NM, L_, B_)[:, :, :A_]
    sw = sliding_window_view(s, K_, axis=1)          # (NM, 28, 20, 3)
    X = sw.transpose(3, 2, 0, 1).reshape(A_ * K_, NM, LOUT)
    X = np.concatenate([X, np.ones((1, NM, LOUT), np.float32)], axis=0)

    Wt = np.asarray(weight, np.float32).transpose(2, 1, 0).reshape(A_ * K_, F_)
    Wb = np.concatenate([Wt, np.asarray(bias, np.float32)[None, :]], axis=0)
    wt = np.zeros((128, F_), np.float32)
    wt[0:KC] = Wb
    wt[64:64 + KC] = Wb

    G = np.zeros((MT, 8 * 32), np.float16)
    for oi in range(8):
        for n in range(NMG):
            G[n * LOUT:(n + 1) * LOUT, 32 * oi + 4 * oi + n] = 1.0

    in_maps = []
    for c in range(CORES):
        Xc = X[:, c * NMC:(c + 1) * NMC, :].reshape(KC, NT, MT)
        xin = np.zeros((128, NT // 2, MT), np.float32)
        xin[0:KC] = Xc[:, 0::2]
        xin[64:64 + KC] = Xc[:, 1::2]
        in_maps.append({
            "xin": np.ascontiguousarray(xin.reshape(128, NT // 2 * MT)),
            "wt": wt,
            "g": G,
        })
    return in_maps


def run_bass(seqs, weight, bias, trace=False):
    """Returns (out (32,32,8000) float32, exec_time_ns or None)."""
    nc = _get_program()
    in_maps = _host_prep(seqs, weight, bias)
    res = run_bass_kernel_spmd(nc, in_maps, list(range(CORES)), trace=trace)
    out = np.concatenate([res.results[c]["out"] for c in range(CORES)], axis=0)
    return out.reshape(N_, M_, F_), res.exec_time_ns


def kernel(seqs, weight, bias):
    out, _ = run_bass(seqs, weight, bias, trace=False)
    return out


# revision 3
# speedup vs baseline: 1.0636x; 1.0636x over previous
"""Trainium2 Bass kernel for nn_KmerEmbed: conv1d(one-hot kmer filters) + relu + window-sum.

Computes, for seqs (32,32,30,21), weight (8000,20,3), bias (8000,):
  out[n,m,f] = sum_l relu( sum_{a,j} seqs[n,m,l+j,a(<20)]*weight[f,a,j] + bias[f] )
with l over the 28 valid conv positions; returns (32,32,8000) float32.

Strategy (8 NeuronCores, data-parallel over the 1024 flattened rows, 128 rows/core):
  - im2col on host: X[(j,a)+bias_row, tile, (n4,l28)] -> stationary operand of a
    K=61 matmul streamed against the replicated filter matrix Wt (61,8000) in
    float32r (1 cycle/row at N>=256, ~1e-4 rel precision).
  - conv tiles are packed in pairs into PE row-groups (partitions 0-60 / 64-124)
    so two matmuls stream concurrently.
  - relu(conv) evaluated from PSUM by ScalarE (activation Relu) and VectorE
    (tensor_scalar max) in parallel, written to SBUF as float16.
  - window-sum via a second matmul with 0/1 selection matrices G (112,32) in
    fp16, one PE column-group per 32-row output block; 8 tiles accumulate into
    each column group of a single (128, chunk) PSUM tile, so the final
    PSUM->SBUF copy covers all 128 partitions at once.
  - staging (128,8000) in SBUF, contiguous DMA to DRAM.
"""

import os
import sys

import numpy as np
from numpy.lib.stride_tricks import sliding_window_view

for _p in ("/opt/trn_rl_repo", "/root/.axon_site/_ro/trn_rl_repo"):
    if os.path.isdir(_p) and _p not in sys.path:
        sys.path.insert(0, _p)

import concourse.bacc as bacc
import concourse.mybir as mybir
from concourse.tile import TileContext
from concourse.bass_utils import run_bass_kernel_spmd

# problem sizes (hardcoded per spec)
N_, M_, L_, B_ = 32, 32, 30, 21
A_, K_ = 20, 3
F_ = 8000
NM = N_ * M_              # 1024
CORES = 8
NMC = NM // CORES         # 128 rows per core
LOUT = L_ - K_ + 1        # 28
NMG = 4                   # rows per conv tile
MT = NMG * LOUT           # 112 psum partitions per conv tile
NT = NMC // NMG           # 32 tiles per core
KC = A_ * K_ + 1          # 61 = 60 + bias row
FCH = 500                 # matmul free-dim chunk (one PSUM bank)
BIG = 1000                # relu/copy chunk (two banks)
NBIG = F_ // BIG          # 8

_f32r = mybir.dt.float32r
_f32 = mybir.dt.float32
_f16 = mybir.dt.float16

_cached_nc = None


def _build_program():
    nc = bacc.Bacc("TRN2", target_bir_lowering=False, debug=False,
                   num_devices=CORES)
    xin_d = nc.declare_dram_parameter("xin", [128, NT // 2 * MT], _f16,
                                      isOutput=False)
    wt_d = nc.declare_dram_parameter("wt", [128, F_], _f16, isOutput=False)
    g_d = nc.declare_dram_parameter("g", [MT, 8 * 32], _f16, isOutput=False)
    out_d = nc.declare_dram_parameter("out", [NMC, F_], _f32, isOutput=True)

    relu_fn = mybir.ActivationFunctionType.Relu
    max_op = mybir.AluOpType.max

    with TileContext(nc) as tc:
        with tc.tile_pool(name="const", bufs=1) as cpool, \
             tc.tile_pool(name="rbuf", bufs=18) as rpool, \
             tc.tile_pool(name="stage", bufs=1) as spool, \
             tc.tile_pool(name="pconv", bufs=3, space="PSUM") as pconv, \
             tc.tile_pool(name="psum", bufs=1, space="PSUM") as psump:
            xin_sb = cpool.tile([128, NT // 2 * MT], _f16)
            wt_sb = cpool.tile([128, F_], _f16)
            g_sb = cpool.tile([MT, 8 * 32], _f16)
            stage = spool.tile([NMC, F_], _f32)
            nc.sync.dma_start(out=xin_sb[:], in_=xin_d[:])
            nc.sync.dma_start(out=g_sb[:], in_=g_d[:])
            for i in range(4):
                s = slice(i * (F_ // 4), (i + 1) * (F_ // 4))
                nc.sync.dma_start(out=wt_sb[:, s], in_=wt_d[:, s])

            sum_order = [g0 * 8 + i for i in range(8) for g0 in range(4)]
            first_in_group = set(range(0, NT, 8))

            for c in range(NBIG):
                fsl = slice(c * BIG, (c + 1) * BIG)
                rtiles = {}
                for p in range(NT // 2):
                    pc_e = pconv.tile([MT, 1024], _f32, tag="pc")
                    pc_o = pconv.tile([MT, 1024], _f32, tag="pc")
                    for h in range(2):
                        fs = slice(c * BIG + h * FCH, c * BIG + (h + 1) * FCH)
                        hs = slice(h * 512, h * 512 + FCH)
                        nc.tensor.matmul(
                            out=pc_e[:, hs],
                            lhsT=xin_sb[0:KC, p * MT:(p + 1) * MT],
                            rhs=wt_sb[0:KC, fs], start=True, stop=True)
                        nc.tensor.matmul(
                            out=pc_o[:, hs],
                            lhsT=xin_sb[64:64 + KC, p * MT:(p + 1) * MT],
                            rhs=wt_sb[64:64 + KC, fs], start=True, stop=True)
                    r_e = rpool.tile([MT, 1024], _f16, tag="re")
                    r_o = rpool.tile([MT, 1024], _f16, tag="ro")
                    nc.scalar.activation(out=r_e[:, 0:1012], in_=pc_e[:, 0:1012],
                                         func=relu_fn)
                    nc.vector.tensor_scalar(out=r_o[:, 0:1012], in0=pc_o[:, 0:1012],
                                            scalar1=0.0, scalar2=None,
                                            op0=max_op)
                    rtiles[2 * p] = r_e
                    rtiles[2 * p + 1] = r_o
                ps = psump.tile([128, 1024], _f32)
                for h in range(2):
                    hs = slice(h * 512, h * 512 + FCH)
                    for t in sum_order:
                        grp = t // 8
                        oi = t % 8
                        nc.tensor.matmul(
                            out=ps[32 * grp:32 * grp + 32, hs],
                            lhsT=g_sb[:, 32 * oi:32 * oi + 32],
                            rhs=rtiles[t][:, hs],
                            start=(t in first_in_group),
                            stop=(t == sum_order[-1]),
                            skip_group_check=True,
                            tile_position=(0, 32 * grp))
                for h in range(2):
                    nc.vector.tensor_copy(
                        out=stage[:, c * BIG + h * FCH:c * BIG + (h + 1) * FCH],
                        in_=ps[:, h * 512:h * 512 + FCH])
                if c % 2 == 1:
                    osl = slice((c - 1) * BIG, (c + 1) * BIG)
                    nc.sync.dma_start(out=out_d[:, osl], in_=stage[:, osl])

    nc.compile()
    return nc


def _get_program():
    global _cached_nc
    if _cached_nc is None:
        _cached_nc = _build_program()
    return _cached_nc


def _host_prep(seqs, weight, bias):
    s = np.asarray(seqs, np.float32).reshape(NM, L_, B_)[:, :, :A_]
    sw = sliding_window_view(s, K_, axis=1)          # (NM, 28, 20, 3)
    X = sw.transpose(3, 2, 0, 1).reshape(A_ * K_, NM, LOUT)
    X = np.concatenate([X, np.ones((1, NM, LOUT), np.float32)], axis=0)

    Wt = np.asarray(weight, np.float32).transpose(2, 1, 0).reshape(A_ * K_, F_)
    Wb = np.concatenate([Wt, np.asarray(bias, np.float32)[None, :]], axis=0)
    wt = np.zeros((128, F_), np.float32)
    wt[0:KC] = Wb
    wt[64:64 + KC] = Wb
    wt_f16 = wt.astype(np.float16)

    G = np.zeros((MT, 8 * 32), np.float16)
    for oi in range(8):
        for n in range(NMG):
            G[n * LOUT:(n + 1) * LOUT, 32 * oi + 4 * oi + n] = 1.0

    in_maps = []
    for c in range(CORES):
        Xc = X[:, c * NMC:(c + 1) * NMC, :].reshape(KC, NT, MT)
        xin = np.zeros((128, NT // 2, MT), np.float32)
        xin[0:KC] = Xc[:, 0::2]
        xin[64:64 + KC] = Xc[:, 1::2]
        in_maps.append({
            "xin": np.ascontiguousarray(xin.reshape(128, NT // 2 * MT)).astype(np.float16),
            "wt": wt_f16,
            "g": G,
        })
    return in_maps


def run_bass(seqs, weight, bias, trace=False):
    """Returns (out (32,32,8000) float32, exec_time_ns or None)."""
    nc = _get_program()
    in_maps = _host_prep(seqs, weight, bias)
    res = run_bass_kernel_spmd(nc, in_maps, list(range(CORES)), trace=trace)
    out = np.concatenate([res.results[c]["out"] for c in range(CORES)], axis=0)
    return out.reshape(N_, M_, F_), res.exec_time_ns


def kernel(seqs, weight, bias):
    out, _ = run_bass(seqs, weight, bias, trace=False)
    return out
